# revision 1
# baseline (speedup 1.0000x reference)
"""ARX recurrence kernel for Trainium2 (8 NeuronCores, data-parallel).

Math: the reference runs out[:, t] = window @ w_ar + (u @ w_u + w_b) as a
sequential scan over 1008 steps.  Since the recurrence is linear, every
output timestep is a linear functional of X = [y | u | 1]:

    out[:, t] = X @ G[:, t]          G: [32, 1024]

G is computed ON DEVICE from w by log-doubling of the (augmented 17x17)
companion matrix: ~10 tiny matmuls.  The bulk work is then a single
[128, 32] x [32, 1024] matmul per 128-row batch tile, executed with 4x
row-tiling of the PE array (K=32 per quadrant), which makes the kernel
output-DMA-bound (32 MB/core).

Batch <-> partition mapping: partition q of batch-tile s holds batch row
64*q + s (so the 512 KB y / 480 KB u inputs load as fully contiguous 4 KB
per-partition DMA chunks, and output rows are still contiguous 4 KB rows).
"""

import numpy as np

import concourse.bacc as bacc
import concourse.bass as bass
import concourse.mybir as mybir
import concourse.tile as tile
from concourse.masks import make_identity
from concourse.bass_utils import run_bass_kernel_spmd

N_CORES = 8
B_FULL = 65536
AR = 16          # ar order
NU = 15          # exogenous dim
K = 32           # regressor dim = AR + NU + 1
S = 1024         # sequence length
T_PRED = S - AR  # 1008 predicted steps

B = B_FULL // N_CORES      # 8192 rows per core
NTILES = B // 128          # 64 batch tiles of 128 rows
GROUPS = NTILES // 4       # 16 groups of 4 tiles (one 128x128 transpose each)
N_CHUNKS = 4               # input loaded in 4 chunks for pipelining
F32 = mybir.dt.float32
# dtype tag for the main matmuls: float32r = same fp32 bits, streamed at
# 1 cyc/row instead of 4 (TF32-like internal precision).
MM_DTYPE = mybir.dt.float32


def _mm(ap):
    """View an AP in the main-matmul dtype (no-op for plain fp32)."""
    return ap if MM_DTYPE == F32 else ap.bitcast(MM_DTYPE)


# scheduling knobs (module-level so experiments can tweak them).
# Defaults are the measured-fastest config (interleaved rep-slope A/B on
# HW: ~79 us/iter vs ~116 us with 3/3/3 buffers).
X4_BUFS = 6
XT_BUFS = 8
OUT_BUFS = 4
PS_BUFS = 6
PSXT_BUFS = 2
ASM_ENGINE = "gpsimd"   # which engine assembles X4 ([y|u|1] copies)
PROLOGUE = True         # False: skip G computation (timing ablation only)
DO_MM = True            # False: skip main matmuls (timing ablation only)
OUT_RING = "alternate"  # output-DMA ring: "sync", "scalar", or "alternate"
IN_ENGINE = "gpsimd"    # engine issuing input loads (SWDGE keeps HWDGE free)
LAYOUT = "quad"         # "quad": one 128x128 transpose + tile_position
                        # "flat": per-tile transposes, all operands base-0
MM_SPLIT = False        # flat only: X=Xhi+Xlo, G=Ghi+Glo, 3 f32r matmuls
                        # (fp32-class accuracy at ~3/4 the fp32 PE cost)
OUT_SPAN = 1            # groups per output DMA (1 -> 2MB stores, 2 -> 4MB)
OUT_DUAL = True         # two 1MB stores per group, one on each ring
                        # (measured 102 us/iter vs 131 single-ring-alternate)
COPY_53 = True          # split psum drains 5 DVE / 3 ACT — balances drain
                        # latency; required for OUT_DUAL to win
G_HALF_SPLIT = True     # release G's first column half one round earlier


def _emit_g_prologue(nc, pools, w_ap, ident):
    """Emit device-side computation of G (the [32, 1024] coefficient
    matrix) into G_rep[0:32], then replicate across the 4 partition
    quadrants.  Returns the G_rep tile."""
    singles, ps_pool, psxt_pool = pools

    # --- companion matrix A [17,17] -------------------------------------
    # rows 0..15 = window-basis shift; row 15 = w_aug; row 16 = const row.
    # Engine ops must start at partition 0, so diagonals are written with
    # affine_select (fill where the affine expr == 0).
    A = singles.tile([17, 17], F32, tag="Amat")
    nc.gpsimd.memset(A[:, :], 0.0)
    # superdiagonal A[x, x+1] = 1 for x=0..15 (incl. A[15,16]=1, wanted):
    # expr = x - y + 1
    nc.gpsimd.affine_select(
        out=A[:, :], in_=A[:, :], compare_op=mybir.AluOpType.not_equal,
        fill=1.0, base=1, pattern=[[-1, 17]], channel_multiplier=1)
    # A[16,16] = 1: expr = x + y - 32 (only zero at (16,16) in range)
    nc.gpsimd.affine_select(
        out=A[:, :], in_=A[:, :], compare_op=mybir.AluOpType.not_equal,
        fill=1.0, base=-32, pattern=[[1, 17]], channel_multiplier=1)
    nc.scalar.dma_start(out=A[15:16, 0:16], in_=w_ap[None, 0:16])

    # AT = A^T via PE transpose
    ps_a = psxt_pool.tile([17, 17], F32, tag="psxt")
    nc.tensor.transpose(ps_a[:, :], A[:, :], ident[0:17, 0:17])
    AT = singles.tile([17, 17], F32, tag="ATmat")
    nc.vector.tensor_copy(out=AT[:, :], in_=ps_a[:, :])

    # T/TT working copies (overwritten every doubling round)
    T = singles.tile([17, 17], F32, tag="Tmat")
    TT = singles.tile([17, 17], F32, tag="TTmat")
    nc.vector.tensor_copy(out=T[:, :], in_=A[:, :])
    nc.vector.tensor_copy(out=TT[:, :], in_=AT[:, :])

    # BT [17, 1024]: column t holds phi_{16+t} (coeffs of out_{16+t} over
    # [y_0..y_15, const]).  BT[:, 0] = [w_ar, 1].
    BT = singles.tile([17, S], F32, tag="BTmat")
    nc.gpsimd.memset(BT[:, :], 0.0)
    # BT[16, 0] = 1: expr = x + 17*y - 16 (zero only at (16,0) since x<17)
    nc.gpsimd.affine_select(
        out=BT[:, :], in_=BT[:, :], compare_op=mybir.AluOpType.not_equal,
        fill=1.0, base=-16, pattern=[[17, S]], channel_multiplier=1)
    nc.scalar.dma_start(out=BT[0:16, 0:1], in_=w_ap[0:16, None])

    # --- doubling: BT[:, L:2L] = TT @ BT[:, :L]; T <- T@T; TT <- TT@TT ---
    L = 1
    while L < S:
        ps_b = ps_pool.tile([17, min(L, 512)], F32, tag="ps")
        if L <= 512:
            nc.tensor.matmul(ps_b[:, 0:L], T[:, :], BT[:, 0:L],
                             start=True, stop=True)
            nc.vector.tensor_copy(out=BT[:, L : 2 * L], in_=ps_b[:, 0:L])
        else:
            # L = 512 -> two chunks (psum bank holds max 512 fp32)
            ps_b2 = ps_pool.tile([17, 512], F32, tag="ps")
            nc.tensor.matmul(ps_b[:, :], T[:, :], BT[:, 0:512],
                             start=True, stop=True)
            nc.tensor.matmul(ps_b2[:, :], T[:, :], BT[:, 512:1024],
                             start=True, stop=True)
            nc.vector.tensor_copy(out=BT[:, L : L + 512], in_=ps_b[:, :])
            nc.vector.tensor_copy(out=BT[:, L + 512 : L + 1024],
                                  in_=ps_b2[:, :])
        L *= 2
        if L < S:  # last round needs no further T/TT update
            ps_t = ps_pool.tile([17, 17], F32, tag="ps")
            ps_tt = ps_pool.tile([17, 17], F32, tag="ps")
            nc.tensor.matmul(ps_t[:, :], TT[:, :], T[:, :],
                             start=True, stop=True)
            nc.tensor.matmul(ps_tt[:, :], T[:, :], TT[:, :],
                             start=True, stop=True)
            nc.vector.tensor_copy(out=T[:, :], in_=ps_t[:, :])
            nc.vector.tensor_copy(out=TT[:, :], in_=ps_tt[:, :])

    # --- assemble G32 into Gtmp [32, :] ---------------------------------
    # Layout: out[:, 0:16] = y (identity block); out[:, 16+t] = X @ phi_t.
    # Write order matters because engine ops must start at partition 0:
    # 1) the [0:32]-partition outer product (zeros on rows 0:16),
    # 2) then overwrite rows 0:16 with the y-coefficients from BT.
    # Assembled in a staging tile, then copied (with fp32r rounding when
    # MM_DTYPE is float32r) into G_rep and replicated across quadrants.
    Gtmp = singles.tile([32, S], F32, tag="Gtmp")
    nc.gpsimd.memset(Gtmp[0:32, 0:16], 0.0)
    make_identity(nc, Gtmp[0:16, 0:16])

    # d row (const coefficients) lives at partition 16 of BT; DMA it to
    # partition 0 so it can be the rhs of the outer-product matmul.
    d_sb = singles.tile([1, T_PRED], F32, tag="drow")
    nc.scalar.dma_start(out=d_sb[:, :], in_=BT[16:17, 0:T_PRED])

    # lhsT = [0]*16 ++ [w_u, w_b]  -> outer product rows land on psum
    # partitions 16..31 (rows 0..15 are zero, overwritten in step 2).
    wub = singles.tile([1, 32], F32, tag="wub")
    nc.vector.memset(wub[:, 0:16], 0.0)
    nc.scalar.dma_start(out=wub[:, 16:32], in_=w_ap[None, 16:32])

    ps_o1 = ps_pool.tile([32, 512], F32, tag="ps")
    ps_o2 = ps_pool.tile([32, 512], F32, tag="ps")
    nc.tensor.matmul(ps_o1[:, :], wub[:, :], d_sb[:, 0:512],
                     start=True, stop=True)
    nc.tensor.matmul(ps_o2[:, 0 : T_PRED - 512], wub[:, :],
                     d_sb[:, 512:T_PRED], start=True, stop=True)
    nc.vector.tensor_copy(out=Gtmp[0:32, AR : AR + 512], in_=ps_o1[:, :])
    nc.vector.tensor_copy(out=Gtmp[0:32, AR + 512 : S],
                          in_=ps_o2[0:32, 0 : T_PRED - 512])
    # step 2: y-coefficient rows overwrite the zero rows of the outer prod.
    # Split at column AR+512 so the first half of G (everything the h=0
    # matmuls read) is ready one doubling round before the last BT append.
    HALF = AR + 512
    if G_HALF_SPLIT:
        nc.vector.tensor_copy(out=Gtmp[0:16, AR:HALF], in_=BT[0:16, 0:512])
        nc.vector.tensor_copy(out=Gtmp[0:16, HALF:S],
                              in_=BT[0:16, 512:T_PRED])
    else:
        nc.vector.tensor_copy(out=Gtmp[0:16, AR:S], in_=BT[0:16, 0:T_PRED])

    if LAYOUT == "quad":
        # --- round into G_rep and replicate across partition quadrants --
        G_rep = singles.tile([128, S], F32, tag="Grep")
        halves = ((0, HALF), (HALF, S)) if G_HALF_SPLIT else ((0, S),)
        for lo, hi in halves:
            nc.vector.tensor_copy(out=_mm(G_rep[0:32, lo:hi]),
                                  in_=Gtmp[0:32, lo:hi])
            for q in range(1, 4):
                nc.scalar.dma_start(
                    out=_mm(G_rep[32 * q : 32 * (q + 1), lo:hi]),
                    in_=_mm(G_rep[0:32, lo:hi]))
        return (G_rep,)
    # --- flat layout: base-0 G (and optional hi/lo split) ---------------
    Ghi = singles.tile([32, S], F32, tag="Ghi")
    nc.vector.tensor_copy(out=_mm(Ghi[:, :]), in_=Gtmp[0:32, :])
    if not MM_SPLIT:
        return (Ghi,)
    Glo = singles.tile([32, S], F32, tag="Glo")
    nc.vector.tensor_tensor(
        out=_mm(Glo[:, :]), in0=Gtmp[0:32, :], in1=Ghi[:, :],
        op=mybir.AluOpType.subtract)
    return (Ghi, Glo)


def build_nc(b=B, reps=1):
    """Build the per-core Bass program (SPMD: same program, 8 shards).

    reps>1 unrolls the whole main loop multiple times inside one NEFF
    (writes the same outputs each rep) — used only for steady-state HW
    timing, never for grading."""
    ntiles = b // 128
    groups = ntiles // 4
    n_chunks = max(1, min(N_CHUNKS, groups))
    grp_per_chunk = groups // n_chunks
    s_per_part = b // 128  # rows per partition in the pack layout

    nc = bacc.Bacc("TRN2", target_bir_lowering=False, debug=False)

    y_d = nc.dram_tensor("y", [b, AR], F32, kind="ExternalInput").ap()
    u_d = nc.dram_tensor("u", [b, NU], F32, kind="ExternalInput").ap()
    w_d = nc.dram_tensor("w", [K], F32, kind="ExternalInput").ap()
    out_d = nc.dram_tensor("out", [b, S], F32, kind="ExternalOutput").ap()

    # pack views: partition q <-> batch rows [q*s_per_part, (q+1)*s_per_part)
    y_pack = y_d.rearrange("(q s) k -> q (s k)", q=128)    # [128, s_per_part*16]
    u_pack = u_d.rearrange("(q s) k -> q (s k)", q=128)    # [128, s_per_part*15]
    out_v = out_d.rearrange("(q s) t -> q s t", q=128)     # [128, s_per_part, 1024]

    from contextlib import ExitStack
    with tile.TileContext(nc) as tc, ExitStack() as ctx:
        singles = ctx.enter_context(tc.tile_pool(name="singles", bufs=1))
        x4_pool = ctx.enter_context(tc.tile_pool(name="x4", bufs=X4_BUFS))
        xt_pool = ctx.enter_context(tc.tile_pool(name="xt", bufs=XT_BUFS))
        out_pool = ctx.enter_context(tc.tile_pool(name="outsb", bufs=OUT_BUFS))
        ps_pool = ctx.enter_context(
            tc.tile_pool(name="ps", bufs=PS_BUFS, space="PSUM"))
        psxt_pool = ctx.enter_context(
            tc.tile_pool(name="psxt", bufs=PSXT_BUFS, space="PSUM"))

        # identity for PE transposes
        ident = singles.tile([128, 128], F32, tag="ident")
        make_identity(nc, ident[:, :])

        if PROLOGUE:
            G_parts = _emit_g_prologue(
                nc, (singles, ps_pool, psxt_pool), w_d, ident)
        else:  # timing ablation only: garbage G
            gp = singles.tile([128 if LAYOUT == "quad" else 32, S], F32,
                              tag="Grep")
            nc.vector.memset(_mm(gp[:, :]), 0.0)
            G_parts = (gp, gp) if (LAYOUT == "flat" and MM_SPLIT) else (gp,)

        # --- input loads (chunked for pipelining) -----------------------
        ychunks, uchunks = [], []
        ccols_y = grp_per_chunk * 4 * AR   # cols of y_pack per chunk
        ccols_u = grp_per_chunk * 4 * NU
        in_eng = getattr(nc, IN_ENGINE)
        for c in range(n_chunks):
            yc = singles.tile([128, ccols_y], F32, tag=f"ypack{c}")
            in_eng.dma_start(
                out=yc[:, :], in_=y_pack[:, c * ccols_y : (c + 1) * ccols_y])
            ychunks.append(yc)
            uc = singles.tile([128, ccols_u], F32, tag=f"upack{c}")
            in_eng.dma_start(
                out=uc[:, :], in_=u_pack[:, c * ccols_u : (c + 1) * ccols_u])
            uchunks.append(uc)

        # --- main loop: one group = 4 batch tiles = one 128x128 transpose
        out_sb = None
        for g in [g for _ in range(reps) for g in range(groups)]:
            c, gl = divmod(g, grp_per_chunk)
            sp = g % OUT_SPAN  # position within the output-DMA span

            # assemble X4 [128, 4, 32] = [y | u | 1] for 4 tiles
            X4 = x4_pool.tile([128, 128], F32, tag="x4")
            x4v = X4[:, :].rearrange("p (a k) -> p a k", a=4)
            yv = ychunks[c][:, gl * 4 * AR : (gl + 1) * 4 * AR].rearrange(
                "p (a k) -> p a k", a=4)
            uv = uchunks[c][:, gl * 4 * NU : (gl + 1) * 4 * NU].rearrange(
                "p (a k) -> p a k", a=4)
            asm = getattr(nc, ASM_ENGINE)
            asm.tensor_copy(out=x4v[:, :, 0:AR], in_=yv)
            asm.tensor_copy(out=x4v[:, :, AR : AR + NU], in_=uv)
            asm.memset(x4v[:, :, K - 1 : K], 1.0)

            if sp == 0:
                out_sb = out_pool.tile([128, OUT_SPAN * 4 * S], F32,
                                       tag="outsb")
            base = sp * 4 * S
            if LAYOUT == "quad":
                # transpose -> XT4 [128,128]: rows 32j..32j+31 = X_j^T
                ps_xt = psxt_pool.tile([128, 128], F32, tag="psxt")
                nc.tensor.transpose(ps_xt[:, :], X4[:, :], ident[:, :])
                XT4 = xt_pool.tile([128, 128], F32, tag="xt")
                nc.vector.tensor_copy(out=_mm(XT4[:, :]), in_=ps_xt[:, :])
                (G_rep,) = G_parts

                # 8 row-tiled matmuls (4 quadrants x 2 column halves)
                for j in range(4):
                    for h in range(2):
                        ps = ps_pool.tile([128, 512], F32, tag="ps")
                        if DO_MM:
                            nc.tensor.matmul(
                                ps[:, :],
                                _mm(XT4[32 * j : 32 * (j + 1), :]),
                                _mm(G_rep[32 * j : 32 * (j + 1),
                                          512 * h : 512 * (h + 1)]),
                                start=True, stop=True,
                                tile_position=(32 * j, 0),
                            )
                        else:
                            nc.vector.memset(ps[:, :], 0.0)
                        dst = out_sb[:, base + j * S + 512 * h
                                     : base + j * S + 512 * (h + 1)]
                        if (j + h) % 2 == 0:
                            nc.vector.tensor_copy(out=dst, in_=ps[:, :])
                        else:
                            nc.scalar.copy(out=dst, in_=ps[:, :])
            else:
                # flat: per-tile [128,32] transpose -> base-0 XT [32,128]
                for j in range(4):
                    ps_xt = psxt_pool.tile([32, 128], F32, tag="psxt")
                    nc.tensor.transpose(
                        ps_xt[:, :], X4[:, 32 * j : 32 * (j + 1)],
                        ident[:, :])
                    XThi = xt_pool.tile([32, 128], F32, tag="xthi")
                    nc.vector.tensor_copy(out=_mm(XThi[:, :]),
                                          in_=ps_xt[:, :])
                    if MM_SPLIT:
                        XTlo = xt_pool.tile([32, 128], F32, tag="xtlo")
                        nc.vector.tensor_tensor(
                            out=_mm(XTlo[:, :]), in0=ps_xt[:, :],
                            in1=XThi[:, :], op=mybir.AluOpType.subtract)
                    for h in range(2):
                        ps = ps_pool.tile([128, 512], F32, tag="ps")
                        cols = slice(512 * h, 512 * (h + 1))
                        if MM_SPLIT:
                            Ghi, Glo = G_parts
                            nc.tensor.matmul(
                                ps[:, :], _mm(XThi[:, :]),
                                _mm(Ghi[:, cols]), start=True, stop=False)
                            nc.tensor.matmul(
                                ps[:, :], _mm(XThi[:, :]),
                                _mm(Glo[:, cols]), start=False, stop=False)
                            nc.tensor.matmul(
                                ps[:, :], _mm(XTlo[:, :]),
                                _mm(Ghi[:, cols]), start=False, stop=True)
                        else:
                            (Ghi,) = G_parts
                            nc.tensor.matmul(
                                ps[:, :], _mm(XThi[:, :]),
                                _mm(Ghi[:, cols]), start=True, stop=True)
                        dst = out_sb[:, base + j * S + 512 * h
                                     : base + j * S + 512 * (h + 1)]
                        idx = j * 2 + h
                        on_dve = (idx < 5) if COPY_53 else ((j + h) % 2 == 0)
                        if on_dve:
                            nc.vector.tensor_copy(out=dst, in_=ps[:, :])
                        else:
                            nc.scalar.copy(out=dst, in_=ps[:, :])

            if OUT_DUAL:
                # two 1MB stores per group, one on each HWDGE ring, so both
                # rings stay busy every group
                sbv = out_sb[:, :].rearrange("p (a t) -> p a t", a=4)
                nc.sync.dma_start(
                    out=out_v[:, 4 * g : 4 * g + 2, :], in_=sbv[:, 0:2, :])
                nc.scalar.dma_start(
                    out=out_v[:, 4 * g + 2 : 4 * g + 4, :],
                    in_=sbv[:, 2:4, :])
            elif sp == OUT_SPAN - 1:
                # one output DMA per OUT_SPAN groups; alternate between the
                # two physical HWDGE rings (SP and ACT)
                if OUT_RING == "alternate":
                    out_eng = nc.sync if (g // OUT_SPAN) % 2 == 0 \
                        else nc.scalar
                else:
                    out_eng = (getattr(nc, OUT_RING)
                               if OUT_RING != "sync" else nc.sync)
                g0 = g - (OUT_SPAN - 1)
                out_eng.dma_start(
                    out=out_v[:, 4 * g0 : 4 * (g + 1), :],
                    in_=out_sb[:, :].rearrange(
                        "p (a t) -> p a t", a=OUT_SPAN * 4),
                )

    nc.compile()
    return nc


_NC_CACHE = {}


def _get_nc(b):
    if b not in _NC_CACHE:
        _NC_CACHE[b] = build_nc(b)
    return _NC_CACHE[b]


def kernel(y, u, w):
    y = np.ascontiguousarray(np.asarray(y), dtype=np.float32)
    u = np.ascontiguousarray(np.asarray(u), dtype=np.float32)
    w = np.ascontiguousarray(np.asarray(w), dtype=np.float32)
    assert y.shape == (B_FULL, AR) and u.shape == (B_FULL, NU)

    nc = _get_nc(B)
    in_maps = [
        {"y": y[i * B : (i + 1) * B], "u": u[i * B : (i + 1) * B], "w": w}
        for i in range(N_CORES)
    ]
    res = run_bass_kernel_spmd(nc, in_maps, list(range(N_CORES)))
    return np.concatenate(
        [res.results[i]["out"] for i in range(N_CORES)], axis=0)



# revision 4
# speedup vs baseline: 1.1306x; 1.1306x over previous
"""ARX recurrence kernel for Trainium2 (8 NeuronCores, data-parallel).

Math: the reference runs out[:, t] = window @ w_ar + (u @ w_u + w_b) as a
sequential scan over 1008 steps.  Since the recurrence is linear, every
output timestep is a linear functional of X = [y | u | 1]:

    out[:, t] = X @ G[:, t]          G: [32, 1024]

G depends only on the 32-element weight vector, so it is computed on the
HOST in float64 (exact to fp32 working precision) and shipped to the
device as a 512 KB input, pre-replicated across the 4 partition
quadrants.  This removes the ~25 us serial on-device prologue that
previously delayed the first output DMA to t=33us.

The bulk work is a single [128, 32] x [32, 1024] matmul per 128-row
batch tile, executed as float32r (1 cyc/row on the PE instead of fp32's
4) with 4x row-tiling of the PE array (K=32 per quadrant), which makes
the kernel output-DMA-bound (32 MB/core at ~360 GB/s -> 93 us floor).

Batch <-> partition mapping: partition q of batch-tile s holds batch row
64*q + s (so the 512 KB y / 480 KB u inputs load as fully contiguous 4 KB
per-partition DMA chunks, and output rows are still contiguous 4 KB rows).
"""

import numpy as np

import concourse.bacc as bacc
import concourse.bass as bass
import concourse.mybir as mybir
import concourse.tile as tile
from concourse.masks import make_identity
from concourse.bass_utils import run_bass_kernel_spmd

N_CORES = 8
B_FULL = 65536
AR = 16          # ar order
NU = 15          # exogenous dim
K = 32           # regressor dim = AR + NU + 1
S = 1024         # sequence length
T_PRED = S - AR  # 1008 predicted steps

B = B_FULL // N_CORES      # 8192 rows per core
NTILES = B // 128          # 64 batch tiles of 128 rows
GROUPS = NTILES // 4       # 16 groups of 4 tiles (one 128x128 transpose each)
N_CHUNKS = 4               # input loaded in 4 chunks for pipelining
F32 = mybir.dt.float32
# dtype tag for the main matmuls: float32r = same fp32 bits, streamed at
# 1 cyc/row instead of 4 (TF32-like internal precision).  Measured rel
# err 1.1e-4 on HW vs the 2e-2 gate.
MM_DTYPE = mybir.dt.float32r


def _mm(ap):
    """View an AP in the main-matmul dtype (no-op for plain fp32)."""
    return ap if MM_DTYPE == F32 else ap.bitcast(MM_DTYPE)


# scheduling knobs (module-level so experiments can tweak them).
X4_BUFS = 6
XT_BUFS = 8
OUT_BUFS = 4
PS_BUFS = 6
PSXT_BUFS = 2
ASM_ENGINE = "gpsimd"   # which engine assembles X4 ([y|u|1] copies)
DO_MM = True            # False: skip main matmuls (timing ablation only)
IN_ENGINE = "gpsimd"    # engine issuing input loads (SWDGE keeps HWDGE free)
OUT_DUAL = True         # two 1MB stores per group, one on each HWDGE ring
COPY_53 = True          # split psum drains 5 DVE / 3 ACT


def host_g(w):
    """Compute G [32, S] on the host in float64.

    out[:, 0:AR] = y;  out[:, AR+t] = y @ a_t + (u @ w_u + w_b) * b_t.
    Row layout matches X = [y | u | 1]:
      G[0:16, :]  = y coefficients (identity block for the prefix)
      G[16:31, t] = w_u * b_t
      G[31, t]    = w_b * b_t
    """
    w = np.asarray(w, np.float64)
    w_ar, w_u, w_b = w[:AR], w[AR : AR + NU], w[AR + NU]
    # Wc [AR, AR+1] maps [y, const] -> current window; e_const adds const.
    Wc = np.zeros((AR, AR + 1))
    Wc[:, :AR] = np.eye(AR)
    preds = np.empty((T_PRED, AR + 1))
    for t in range(T_PRED):
        pc = w_ar @ Wc
        pc[AR] += 1.0
        preds[t] = pc
        Wc = np.concatenate([Wc[1:], pc[None, :]], axis=0)
    G = np.zeros((K, S), np.float64)
    G[:AR, :AR] = np.eye(AR)
    G[:AR, AR:] = preds[:, :AR].T
    G[AR : AR + NU, AR:] = np.outer(w_u, preds[:, AR])
    G[K - 1, AR:] = w_b * preds[:, AR]
    return G.astype(np.float32)


def build_nc(b=B, reps=1):
    """Build the per-core Bass program (SPMD: same program, 8 shards).

    reps>1 unrolls the whole main loop multiple times inside one NEFF
    (writes the same outputs each rep) — used only for steady-state HW
    timing, never for grading."""
    ntiles = b // 128
    groups = ntiles // 4
    n_chunks = max(1, min(N_CHUNKS, groups))
    grp_per_chunk = groups // n_chunks
    s_per_part = b // 128  # rows per partition in the pack layout

    nc = bacc.Bacc("TRN2", target_bir_lowering=False, debug=False)

    y_d = nc.dram_tensor("y", [b, AR], F32, kind="ExternalInput").ap()
    u_d = nc.dram_tensor("u", [b, NU], F32, kind="ExternalInput").ap()
    g_d = nc.dram_tensor("g", [128, S], F32, kind="ExternalInput").ap()
    out_d = nc.dram_tensor("out", [b, S], F32, kind="ExternalOutput").ap()

    # pack views: partition q <-> batch rows [q*s_per_part, (q+1)*s_per_part)
    y_pack = y_d.rearrange("(q s) k -> q (s k)", q=128)    # [128, s_per_part*16]
    u_pack = u_d.rearrange("(q s) k -> q (s k)", q=128)    # [128, s_per_part*15]
    out_v = out_d.rearrange("(q s) t -> q s t", q=128)     # [128, s_per_part, 1024]

    from contextlib import ExitStack
    with tile.TileContext(nc) as tc, ExitStack() as ctx:
        singles = ctx.enter_context(tc.tile_pool(name="singles", bufs=1))
        x4_pool = ctx.enter_context(tc.tile_pool(name="x4", bufs=X4_BUFS))
        xt_pool = ctx.enter_context(tc.tile_pool(name="xt", bufs=XT_BUFS))
        out_pool = ctx.enter_context(tc.tile_pool(name="outsb", bufs=OUT_BUFS))
        ps_pool = ctx.enter_context(
            tc.tile_pool(name="ps", bufs=PS_BUFS, space="PSUM"))
        psxt_pool = ctx.enter_context(
            tc.tile_pool(name="psxt", bufs=PSXT_BUFS, space="PSUM"))

        in_eng = getattr(nc, IN_ENGINE)

        # G, host-computed, pre-replicated across the 4 partition quadrants.
        # Split into halves so the h=0 matmuls can start before the h=1
        # columns land.
        G_rep = singles.tile([128, S], F32, tag="Grep")
        in_eng.dma_start(out=_mm(G_rep[:, 0:512]), in_=_mm(g_d[:, 0:512]))
        in_eng.dma_start(out=_mm(G_rep[:, 512:S]), in_=_mm(g_d[:, 512:S]))

        # identity for PE transposes
        ident = singles.tile([128, 128], F32, tag="ident")
        make_identity(nc, ident[:, :])

        # --- input loads (chunked for pipelining) -----------------------
        ychunks, uchunks = [], []
        ccols_y = grp_per_chunk * 4 * AR   # cols of y_pack per chunk
        ccols_u = grp_per_chunk * 4 * NU
        for c in range(n_chunks):
            yc = singles.tile([128, ccols_y], F32, tag=f"ypack{c}")
            in_eng.dma_start(
                out=yc[:, :], in_=y_pack[:, c * ccols_y : (c + 1) * ccols_y])
            ychunks.append(yc)
            uc = singles.tile([128, ccols_u], F32, tag=f"upack{c}")
            in_eng.dma_start(
                out=uc[:, :], in_=u_pack[:, c * ccols_u : (c + 1) * ccols_u])
            uchunks.append(uc)

        # --- main loop: one group = 4 batch tiles = one 128x128 transpose
        for g in [g for _ in range(reps) for g in range(groups)]:
            c, gl = divmod(g, grp_per_chunk)

            # assemble X4 [128, 4, 32] = [y | u | 1] for 4 tiles
            X4 = x4_pool.tile([128, 128], F32, tag="x4")
            x4v = X4[:, :].rearrange("p (a k) -> p a k", a=4)
            yv = ychunks[c][:, gl * 4 * AR : (gl + 1) * 4 * AR].rearrange(
                "p (a k) -> p a k", a=4)
            uv = uchunks[c][:, gl * 4 * NU : (gl + 1) * 4 * NU].rearrange(
                "p (a k) -> p a k", a=4)
            asm = getattr(nc, ASM_ENGINE)
            asm.tensor_copy(out=x4v[:, :, 0:AR], in_=yv)
            asm.tensor_copy(out=x4v[:, :, AR : AR + NU], in_=uv)
            asm.memset(x4v[:, :, K - 1 : K], 1.0)

            out_sb = out_pool.tile([128, 4 * S], F32, tag="outsb")

            # transpose -> XT4 [128,128]: rows 32j..32j+31 = X_j^T
            ps_xt = psxt_pool.tile([128, 128], F32, tag="psxt")
            nc.tensor.transpose(ps_xt[:, :], X4[:, :], ident[:, :])
            XT4 = xt_pool.tile([128, 128], F32, tag="xt")
            nc.vector.tensor_copy(out=_mm(XT4[:, :]), in_=ps_xt[:, :])

            # 8 row-tiled matmuls (4 quadrants x 2 column halves)
            for j in range(4):
                for h in range(2):
                    ps = ps_pool.tile([128, 512], F32, tag="ps")
                    if DO_MM:
                        nc.tensor.matmul(
                            ps[:, :],
                            _mm(XT4[32 * j : 32 * (j + 1), :]),
                            _mm(G_rep[32 * j : 32 * (j + 1),
                                      512 * h : 512 * (h + 1)]),
                            start=True, stop=True,
                            tile_position=(32 * j, 0),
                        )
                    else:
                        nc.vector.memset(ps[:, :], 0.0)
                    dst = out_sb[:, j * S + 512 * h : j * S + 512 * (h + 1)]
                    idx = j * 2 + h
                    on_dve = (idx < 5) if COPY_53 else ((j + h) % 2 == 0)
                    if on_dve:
                        nc.vector.tensor_copy(out=dst, in_=ps[:, :])
                    else:
                        nc.scalar.copy(out=dst, in_=ps[:, :])

            if OUT_DUAL:
                # two 1MB stores per group, one on each HWDGE ring, so both
                # rings stay busy every group
                sbv = out_sb[:, :].rearrange("p (a t) -> p a t", a=4)
                nc.sync.dma_start(
                    out=out_v[:, 4 * g : 4 * g + 2, :], in_=sbv[:, 0:2, :])
                nc.scalar.dma_start(
                    out=out_v[:, 4 * g + 2 : 4 * g + 4, :],
                    in_=sbv[:, 2:4, :])
            else:
                out_eng = nc.sync if g % 2 == 0 else nc.scalar
                out_eng.dma_start(
                    out=out_v[:, 4 * g : 4 * (g + 1), :],
                    in_=out_sb[:, :].rearrange("p (a t) -> p a t", a=4),
                )

    nc.compile()
    return nc


_NC_CACHE = {}


def _get_nc(b):
    if b not in _NC_CACHE:
        _NC_CACHE[b] = build_nc(b)
    return _NC_CACHE[b]


def make_in_maps(y, u, w):
    """Per-core input dicts for run_bass_kernel_spmd / the slope bench."""
    y = np.ascontiguousarray(np.asarray(y), dtype=np.float32)
    u = np.ascontiguousarray(np.asarray(u), dtype=np.float32)
    w = np.ascontiguousarray(np.asarray(w), dtype=np.float32)
    g32 = host_g(w)                       # [32, S] f32
    g_rep = np.ascontiguousarray(np.tile(g32, (4, 1)))  # [128, S]
    return [
        {"y": y[i * B : (i + 1) * B], "u": u[i * B : (i + 1) * B],
         "g": g_rep}
        for i in range(N_CORES)
    ]


def kernel(y, u, w):
    assert np.asarray(y).shape == (B_FULL, AR)
    assert np.asarray(u).shape == (B_FULL, NU)
    nc = _get_nc(B)
    in_maps = make_in_maps(y, u, w)
    res = run_bass_kernel_spmd(nc, in_maps, list(range(N_CORES)))
    return np.concatenate(
        [res.results[i]["out"] for i in range(N_CORES)], axis=0)
